# revision 5
# baseline (speedup 1.0000x reference)
"""MoE sparse layer (dense all-expert compute, top-2 gating) on 8 TRN2 cores.

Sharding: token-parallel. x has 8192 tokens; each core processes 1024 tokens
against all 8 experts (weights replicated). No collectives.

Per-core device program (T=1024 tokens, D=1024, E=8, F=4096):
  gating:  logits[t,e] = x @ gate_w + gate_b   (fp32 matmuls, K-tiled)
           G[t,e] = softmax(logits)[t,e] * [e in top2(t)]   (DVE/ACT small ops)
  experts: for e in 0..7, token-halves of 512:
           hT[f,t] = gelu(w1[e].T-tiles @ xT + b1[e])       (fp16 MMs, ACT gelu)
           y[t,d]  = hT.T-tiles @ w2[e] + b2[e]             (fp16 MMs, K=1 bias MM)
           acc[t,d] (+)= G[t,e] * y[t,d]                    (DVE scalar_tensor_tensor)
Layouts are host-prepped so every DMA is a contiguous [128, X] panel.
"""
import numpy as np
import ml_dtypes
from contextlib import ExitStack

import concourse.bacc as bacc_mod
import concourse.tile as tile
import concourse.mybir as mybir

F32 = mybir.dt.float32
F16 = mybir.dt.float16
AF = mybir.ActivationFunctionType
ALU = mybir.AluOpType

N_CORES = 8
HIDDEN = 1024          # D
NUM_EXPERTS = 8        # E
EXPERT_HIDDEN = 4096   # F
TOP_K = 2
TC = 1024              # tokens per core
NKT = HIDDEN // 128    # 8 K-tiles over D
NJ = EXPERT_HIDDEN // 128  # 32 f-tiles
NTI = TC // 128        # 8 token tiles


def build_nc(dbg=False):
    nc = bacc_mod.Bacc("TRN2", target_bir_lowering=False, debug=False)

    xT = nc.dram_tensor("xT", [HIDDEN, TC], F32, kind="ExternalInput").ap()
    w1p = nc.dram_tensor("w1p", [NUM_EXPERTS, NJ, 128, NKT * 128], F16, kind="ExternalInput").ap()
    w2p = nc.dram_tensor("w2p", [NUM_EXPERTS, NJ, 128, HIDDEN], F16, kind="ExternalInput").ap()
    gwp = nc.dram_tensor("gwp", [128, NKT, NUM_EXPERTS], F32, kind="ExternalInput").ap()
    gbp = nc.dram_tensor("gbp", [1, NUM_EXPERTS], F32, kind="ExternalInput").ap()
    b1p = nc.dram_tensor("b1p", [128, NUM_EXPERTS * NJ], F32, kind="ExternalInput").ap()
    b2p = nc.dram_tensor("b2p", [NUM_EXPERTS, HIDDEN], F16, kind="ExternalInput").ap()
    out = nc.dram_tensor("out", [TC, HIDDEN], F32, kind="ExternalOutput").ap()
    if dbg:
        dbglg = nc.dram_tensor("dbglg", [128, NTI * NUM_EXPERTS], F32, kind="ExternalOutput").ap()
        dbgG = nc.dram_tensor("dbgG", [128, NTI * NUM_EXPERTS], F32, kind="ExternalOutput").ap()

    with tile.TileContext(nc) as tc, ExitStack() as ctx:
        # ---- persistent SBUF pools (bytes/partition) ----
        pers = ctx.enter_context(tc.tile_pool(name="pers", bufs=1))
        x16 = pers.tile([128, NKT, TC], F16, tag="x16")             # 16KB
        hT = pers.tile([128, NJ, 512], F16, tag="hT")               # 32KB (token half)
        w2t = pers.tile([128, NJ, HIDDEN], F16, tag="w2t")          # 64KB
        acc = pers.tile([128, NTI, HIDDEN], F32, tag="acc")         # 32KB
        G = pers.tile([128, NTI, NUM_EXPERTS], F32, tag="G")        # 256B
        b1sb = pers.tile([128, NUM_EXPERTS * NJ], F32, tag="b1sb")  # 1KB
        gwsb = pers.tile([128, NKT, NUM_EXPERTS], F32, tag="gwsb")  # 256B
        b2sb = pers.tile([1, NUM_EXPERTS * HIDDEN], F16, tag="b2sb")  # 16KB (1 partition row)
        gbsb = pers.tile([1, NUM_EXPERTS], F32, tag="gbsb")
        ones16 = pers.tile([1, 128], F16, tag="ones16")
        ones32 = pers.tile([1, 128], F32, tag="ones32")

        nc.sync.dma_start(b1sb[:], b1p)
        nc.sync.dma_start(gwsb[:], gwp)
        nc.sync.dma_start(b2sb[:], b2p.rearrange("e d -> (e d)").unsqueeze(0))
        nc.sync.dma_start(gbsb[:], gbp)
        nc.gpsimd.memset(ones16[:], 1.0)
        nc.gpsimd.memset(ones32[:], 1.0)

        # ---- gating (transient pools; close before expert phase) ----
        with tc.tile_pool(name="gat", bufs=1) as gat, \
             tc.tile_pool(name="gtmp", bufs=10) as gtmp, \
             tc.tile_pool(name="gps", bufs=1, space="PSUM") as gpsp:
            gps = gpsp.tile([128, NTI, NUM_EXPERTS], F32, tag="gps")  # one bank
            xg = gat.tile([128, NKT, TC], F32, tag="xg")
            nc.sync.dma_start(xg[:], xT.rearrange("(kt p) t -> p kt t", p=128))
            nc.vector.tensor_copy(x16[:], xg[:])  # fp32 -> fp16 cast
            # NOTE: each token tile's accumulation group must be fully
            # sequential within the shared PSUM bank (start=True clears
            # has_written bank-wide).
            for ti in range(NTI):
                for kt in range(NKT):
                    nc.tensor.matmul(gps[:, ti, :], xg[:, kt, ti * 128:(ti + 1) * 128],
                                     gwsb[:, kt, :], start=(kt == 0), stop=False)
                nc.tensor.matmul(gps[:, ti, :], ones32[0:1, :], gbsb[0:1, :],
                                 start=False, stop=True)
            if dbg:
                lgcopy = gat.tile([128, NTI * NUM_EXPERTS], F32, tag="lgcopy")
                nc.vector.tensor_copy(lgcopy[:], gps[:, :, :])
                nc.sync.dma_start(dbglg, lgcopy[:])
            # softmax + top-2 mask per token tile
            for ti in range(NTI):
                lg = gps[:, ti, :]
                mx = gtmp.tile([128, 1], F32, tag="mx")
                nc.vector.reduce_max(mx[:], lg, axis=mybir.AxisListType.X)
                nmx = gtmp.tile([128, 1], F32, tag="nmx")
                nc.vector.tensor_scalar_mul(nmx[:], mx[:], -1.0)
                ex = gtmp.tile([128, NUM_EXPERTS], F32, tag="ex")
                s = gtmp.tile([128, 1], F32, tag="s")
                nc.scalar.activation(ex[:], lg, AF.Exp, bias=nmx[:], accum_out=s[:])
                r = gtmp.tile([128, 1], F32, tag="r")
                nc.vector.reciprocal(r[:], s[:])
                m1 = gtmp.tile([128, 1], F32, tag="m1")
                nc.vector.reduce_max(m1[:], ex[:], axis=mybir.AxisListType.X)
                eq = gtmp.tile([128, NUM_EXPERTS], F32, tag="eq")
                nc.vector.tensor_scalar(eq[:], ex[:], m1[:], None, op0=ALU.is_equal)
                ex2 = gtmp.tile([128, NUM_EXPERTS], F32, tag="ex2")
                nc.vector.scalar_tensor_tensor(ex2[:], eq[:], -1e9, ex[:],
                                               op0=ALU.mult, op1=ALU.add)
                m2 = gtmp.tile([128, 1], F32, tag="m2")
                nc.vector.reduce_max(m2[:], ex2[:], axis=mybir.AxisListType.X)
                gm = gtmp.tile([128, NUM_EXPERTS], F32, tag="gm")
                nc.vector.scalar_tensor_tensor(gm[:], ex[:], m2[:], ex[:],
                                               op0=ALU.is_ge, op1=ALU.mult)
                nc.vector.tensor_scalar_mul(G[:, ti, :], gm[:], r[:])

        if dbg:
            nc.sync.dma_start(dbgG, G[:, :, :])
        # ---- expert phase ----
        with tc.tile_pool(name="w1pool", bufs=3) as w1pool, \
             tc.tile_pool(name="ps1", bufs=2, space="PSUM") as ps1p, \
             tc.tile_pool(name="ps2", bufs=6, space="PSUM") as ps2p:
            for e in range(NUM_EXPERTS):
                nc.sync.dma_start(w2t[:], w2p[e].rearrange("j p d -> p j d"))
                for half in range(2):
                    tbase = half * 512
                    # mm1: hT[f, t_half] = gelu(w1[e].T @ x + b1[e])
                    for j in range(NJ):
                        w1pan = w1pool.tile([128, NKT * 128], F16, tag="w1pan")
                        nc.sync.dma_start(w1pan[:], w1p[e, j])
                        ps1 = ps1p.tile([128, 512], F32, tag="ps1")
                        for kt in range(NKT):
                            nc.tensor.matmul(
                                ps1[:], w1pan[:, kt * 128:(kt + 1) * 128],
                                x16[:, kt, tbase:tbase + 512],
                                start=(kt == 0), stop=(kt == NKT - 1))
                        nc.scalar.activation(hT[:, j, :], ps1[:], AF.Gelu,
                                             bias=b1sb[:, e * NJ + j:e * NJ + j + 1])
                    # mm2: y[t_tile, d] += hT.T @ w2 + b2; combine into acc.
                    # One PSUM accumulation group at a time: 32 consecutive
                    # same-bank MMs (bank switches are expensive on PE).
                    for ti_loc in range(4):
                        ti = half * 4 + ti_loc
                        for dh in range(2):
                            ps2 = ps2p.tile([128, 512], F32, tag="ps2")
                            for j in range(NJ):
                                nc.tensor.matmul(
                                    ps2[:],
                                    hT[:, j, ti_loc * 128:(ti_loc + 1) * 128],
                                    w2t[:, j, dh * 512:dh * 512 + 512],
                                    start=(j == 0), stop=False)
                            nc.tensor.matmul(
                                ps2[:], ones16[0:1, :],
                                b2sb[0:1, e * HIDDEN + dh * 512:e * HIDDEN + dh * 512 + 512],
                                start=False, stop=True)
                            gsl = G[:, ti, e:e + 1]
                            asl = acc[:, ti, dh * 512:dh * 512 + 512]
                            if e == 0:
                                nc.vector.tensor_scalar_mul(asl, ps2[:], gsl)
                            else:
                                nc.vector.scalar_tensor_tensor(
                                    asl, ps2[:], gsl, asl,
                                    op0=ALU.mult, op1=ALU.add)
        # ---- store ----
        for ti in range(NTI):
            nc.sync.dma_start(out[ti * 128:(ti + 1) * 128, :], acc[:, ti, :])

    nc.compile()
    return nc


def prep_weights(gate_w, gate_b, w1, b1, w2, b2):
    E, D, F = NUM_EXPERTS, HIDDEN, EXPERT_HIDDEN
    w1p = np.ascontiguousarray(
        w1.reshape(E, NKT, 128, NJ, 128).transpose(0, 3, 2, 1, 4).reshape(E, NJ, 128, NKT * 128)
    ).astype(np.float16)
    w2p = np.ascontiguousarray(w2.reshape(E, NJ, 128, D)).astype(np.float16)
    gwp = np.ascontiguousarray(gate_w.reshape(NKT, 128, E).transpose(1, 0, 2))
    gbp = np.ascontiguousarray(gate_b.reshape(1, E))
    b1p = np.ascontiguousarray(b1.reshape(E, NJ, 128).transpose(2, 0, 1).reshape(128, E * NJ))
    b2p = b2.astype(np.float16)
    return dict(w1p=w1p, w2p=w2p, gwp=gwp, gbp=gbp, b1p=b1p, b2p=b2p)


def make_in_maps(x, gate_w, gate_b, w1, b1, w2, b2):
    x2d = np.asarray(x, dtype=np.float32).reshape(-1, HIDDEN)
    shared = prep_weights(np.asarray(gate_w, np.float32), np.asarray(gate_b, np.float32),
                          np.asarray(w1, np.float32), np.asarray(b1, np.float32),
                          np.asarray(w2, np.float32), np.asarray(b2, np.float32))
    in_maps = []
    for c in range(N_CORES):
        xs = x2d[c * TC:(c + 1) * TC]
        in_maps.append({"xT": np.ascontiguousarray(xs.T), **shared})
    return in_maps


_CACHED_NC = None


def kernel(x, gate_w, gate_b, w1, b1, w2, b2):
    global _CACHED_NC
    from concourse.bass_utils import run_bass_kernel_spmd

    orig_shape = np.asarray(x).shape
    in_maps = make_in_maps(x, gate_w, gate_b, w1, b1, w2, b2)
    if _CACHED_NC is None:
        _CACHED_NC = build_nc()
    res = run_bass_kernel_spmd(_CACHED_NC, in_maps, core_ids=list(range(N_CORES)))
    out = np.concatenate([res.results[c]["out"] for c in range(N_CORES)], axis=0)
    out = out.reshape(orig_shape).astype(np.float32)
    aux_loss = np.asarray(0.0, dtype=np.float32)
    return (out, aux_loss)


# revision 8
# speedup vs baseline: 1.0265x; 1.0265x over previous
"""MoE sparse layer (dense all-expert compute, top-2 gating) on 8 TRN2 cores.

Sharding: token-parallel. x has 8192 tokens; each core processes 1024 tokens
against all 8 experts (weights replicated). No collectives.

Per-core device program (T=1024 tokens, D=1024, E=8, F=4096):
  gating:  logits[t,e] = x @ gate_w + gate_b   (fp32 matmuls, K-tiled)
           G[t,e] = softmax(logits)[t,e] * [e in top2(t)]   (DVE/ACT small ops)
  experts: for e in 0..7, token-halves of 512:
           hT[f,t] = gelu(w1[e].T-tiles @ xT + b1[e])       (fp16 MMs, ACT gelu)
           y[t,d]  = hT.T-tiles @ w2[e] + b2[e]             (fp16 MMs, K=1 bias MM)
           acc[t,d] (+)= G[t,e] * y[t,d]                    (DVE scalar_tensor_tensor)
Layouts are host-prepped so every DMA is a contiguous [128, X] panel.
"""
import numpy as np
from contextlib import ExitStack

import concourse.bacc as bacc_mod
import concourse.tile as tile
import concourse.mybir as mybir

F32 = mybir.dt.float32
F16 = mybir.dt.float16
AF = mybir.ActivationFunctionType
ALU = mybir.AluOpType

N_CORES = 8
HIDDEN = 1024          # D
NUM_EXPERTS = 8        # E
EXPERT_HIDDEN = 4096   # F
TOP_K = 2
TC = 1024              # tokens per core
NKT = HIDDEN // 128    # 8 K-tiles over D
NJ = EXPERT_HIDDEN // 128  # 32 f-tiles
NTI = TC // 128        # 8 token tiles


def build_nc(dbg=False, mm2_interleave=False, split_dma=False):
    nc = bacc_mod.Bacc("TRN2", target_bir_lowering=False, debug=False)

    xT = nc.dram_tensor("xT", [HIDDEN, TC], F32, kind="ExternalInput").ap()
    w1p = nc.dram_tensor("w1p", [NUM_EXPERTS, NJ, 128, NKT * 128], F16, kind="ExternalInput").ap()
    w2p = nc.dram_tensor("w2p", [NUM_EXPERTS, NJ, 128, HIDDEN], F16, kind="ExternalInput").ap()
    gwp = nc.dram_tensor("gwp", [128, NKT, NUM_EXPERTS], F32, kind="ExternalInput").ap()
    gbp = nc.dram_tensor("gbp", [1, NUM_EXPERTS], F32, kind="ExternalInput").ap()
    b1p = nc.dram_tensor("b1p", [128, NUM_EXPERTS * NJ], F32, kind="ExternalInput").ap()
    b2p = nc.dram_tensor("b2p", [NUM_EXPERTS, HIDDEN], F16, kind="ExternalInput").ap()
    out = nc.dram_tensor("out", [TC, HIDDEN], F32, kind="ExternalOutput").ap()
    if dbg:
        dbglg = nc.dram_tensor("dbglg", [128, NTI * NUM_EXPERTS], F32, kind="ExternalOutput").ap()
        dbgG = nc.dram_tensor("dbgG", [128, NTI * NUM_EXPERTS], F32, kind="ExternalOutput").ap()

    with tile.TileContext(nc) as tc, ExitStack() as ctx:
        # ---- persistent SBUF pools (bytes/partition) ----
        pers = ctx.enter_context(tc.tile_pool(name="pers", bufs=1))
        x16 = pers.tile([128, NKT, TC], F16, tag="x16")             # 16KB
        hT = pers.tile([128, NJ, 512], F16, tag="hT")               # 32KB (token half)
        w2t = pers.tile([128, NJ, HIDDEN], F16, tag="w2t")          # 64KB
        acc = pers.tile([128, NTI, HIDDEN], F32, tag="acc")         # 32KB
        G = pers.tile([128, NTI, NUM_EXPERTS], F32, tag="G")        # 256B
        b1sb = pers.tile([128, NUM_EXPERTS * NJ], F32, tag="b1sb")  # 1KB
        gwsb = pers.tile([128, NKT, NUM_EXPERTS], F32, tag="gwsb")  # 256B
        b2sb = pers.tile([1, NUM_EXPERTS * HIDDEN], F16, tag="b2sb")  # 16KB (1 partition row)
        gbsb = pers.tile([1, NUM_EXPERTS], F32, tag="gbsb")
        ones16 = pers.tile([1, 128], F16, tag="ones16")
        ones32 = pers.tile([1, 128], F32, tag="ones32")

        nc.sync.dma_start(b1sb[:], b1p)
        nc.sync.dma_start(gwsb[:], gwp)
        nc.sync.dma_start(b2sb[:], b2p.rearrange("e d -> (e d)").unsqueeze(0))
        nc.sync.dma_start(gbsb[:], gbp)
        nc.gpsimd.memset(ones16[:], 1.0)
        nc.gpsimd.memset(ones32[:], 1.0)

        # ---- gating (transient pools; close before expert phase) ----
        with tc.tile_pool(name="gat", bufs=1) as gat, \
             tc.tile_pool(name="gtmp", bufs=10) as gtmp, \
             tc.tile_pool(name="gps", bufs=1, space="PSUM") as gpsp:
            gps = gpsp.tile([128, NTI, NUM_EXPERTS], F32, tag="gps")  # one bank
            xg = gat.tile([128, NKT, TC], F32, tag="xg")
            nc.sync.dma_start(xg[:], xT.rearrange("(kt p) t -> p kt t", p=128))
            nc.vector.tensor_copy(x16[:], xg[:])  # fp32 -> fp16 cast
            # NOTE: each token tile's accumulation group must be fully
            # sequential within the shared PSUM bank (start=True clears
            # has_written bank-wide).
            for ti in range(NTI):
                for kt in range(NKT):
                    nc.tensor.matmul(gps[:, ti, :], xg[:, kt, ti * 128:(ti + 1) * 128],
                                     gwsb[:, kt, :], start=(kt == 0), stop=False)
                nc.tensor.matmul(gps[:, ti, :], ones32[0:1, :], gbsb[0:1, :],
                                 start=False, stop=True)
            if dbg:
                lgcopy = gat.tile([128, NTI * NUM_EXPERTS], F32, tag="lgcopy")
                nc.vector.tensor_copy(lgcopy[:], gps[:, :, :])
                nc.sync.dma_start(dbglg, lgcopy[:])
            # softmax + top-2 mask per token tile
            for ti in range(NTI):
                lg = gps[:, ti, :]
                mx = gtmp.tile([128, 1], F32, tag="mx")
                nc.vector.reduce_max(mx[:], lg, axis=mybir.AxisListType.X)
                nmx = gtmp.tile([128, 1], F32, tag="nmx")
                nc.vector.tensor_scalar_mul(nmx[:], mx[:], -1.0)
                ex = gtmp.tile([128, NUM_EXPERTS], F32, tag="ex")
                s = gtmp.tile([128, 1], F32, tag="s")
                nc.scalar.activation(ex[:], lg, AF.Exp, bias=nmx[:], accum_out=s[:])
                r = gtmp.tile([128, 1], F32, tag="r")
                nc.vector.reciprocal(r[:], s[:])
                m1 = gtmp.tile([128, 1], F32, tag="m1")
                nc.vector.reduce_max(m1[:], ex[:], axis=mybir.AxisListType.X)
                eq = gtmp.tile([128, NUM_EXPERTS], F32, tag="eq")
                nc.vector.tensor_scalar(eq[:], ex[:], m1[:], None, op0=ALU.is_equal)
                ex2 = gtmp.tile([128, NUM_EXPERTS], F32, tag="ex2")
                nc.vector.scalar_tensor_tensor(ex2[:], eq[:], -1e9, ex[:],
                                               op0=ALU.mult, op1=ALU.add)
                m2 = gtmp.tile([128, 1], F32, tag="m2")
                nc.vector.reduce_max(m2[:], ex2[:], axis=mybir.AxisListType.X)
                gm = gtmp.tile([128, NUM_EXPERTS], F32, tag="gm")
                nc.vector.scalar_tensor_tensor(gm[:], ex[:], m2[:], ex[:],
                                               op0=ALU.is_ge, op1=ALU.mult)
                nc.vector.tensor_scalar_mul(G[:, ti, :], gm[:], r[:])

        if dbg:
            nc.sync.dma_start(dbgG, G[:, :, :])
        # ---- expert phase ----
        with tc.tile_pool(name="w1pool", bufs=3) as w1pool, \
             tc.tile_pool(name="ps1", bufs=2, space="PSUM") as ps1p, \
             tc.tile_pool(name="ps2", bufs=6, space="PSUM") as ps2p:
            for e in range(NUM_EXPERTS):
                if split_dma:
                    for q in range(4):
                        nc.sync.dma_start(
                            w2t[:, q * 8:(q + 1) * 8, :],
                            w2p[e, q * 8:(q + 1) * 8].rearrange("j p d -> p j d"))
                else:
                    nc.sync.dma_start(w2t[:], w2p[e].rearrange("j p d -> p j d"))
                for half in range(2):
                    tbase = half * 512
                    # mm1: hT[f, t_half] = gelu(w1[e].T @ x + b1[e])
                    for j in range(NJ):
                        w1pan = w1pool.tile([128, NKT * 128], F16, tag="w1pan")
                        nc.sync.dma_start(w1pan[:], w1p[e, j])
                        ps1 = ps1p.tile([128, 512], F32, tag="ps1")
                        for kt in range(NKT):
                            nc.tensor.matmul(
                                ps1[:], w1pan[:, kt * 128:(kt + 1) * 128],
                                x16[:, kt, tbase:tbase + 512],
                                start=(kt == 0), stop=(kt == NKT - 1))
                        nc.scalar.activation(hT[:, j, :], ps1[:], AF.Gelu,
                                             bias=b1sb[:, e * NJ + j:e * NJ + j + 1])
                    # mm2: y[t_tile, d] += hT.T @ w2 + b2; combine into acc.
                    # One PSUM accumulation group at a time: 32 consecutive
                    # same-bank MMs (bank switches are expensive on PE).
                    if not mm2_interleave:
                        groups = [(ti_loc, dh) for ti_loc in range(4) for dh in range(2)]
                        for ti_loc, dh in groups:
                            ti = half * 4 + ti_loc
                            ps2 = ps2p.tile([128, 512], F32, tag="ps2")
                            for j in range(NJ):
                                nc.tensor.matmul(
                                    ps2[:],
                                    hT[:, j, ti_loc * 128:(ti_loc + 1) * 128],
                                    w2t[:, j, dh * 512:dh * 512 + 512],
                                    start=(j == 0), stop=False)
                            nc.tensor.matmul(
                                ps2[:], ones16[0:1, :],
                                b2sb[0:1, e * HIDDEN + dh * 512:e * HIDDEN + dh * 512 + 512],
                                start=False, stop=True)
                            gsl = G[:, ti, e:e + 1]
                            asl = acc[:, ti, dh * 512:dh * 512 + 512]
                            if e == 0:
                                nc.vector.tensor_scalar_mul(asl, ps2[:], gsl)
                            else:
                                nc.vector.scalar_tensor_tensor(
                                    asl, ps2[:], gsl, asl,
                                    op0=ALU.mult, op1=ALU.add)
                    else:
                        for tg in range(2):
                            for dh in range(2):
                                pss = []
                                for tt in range(2):
                                    ps2 = ps2p.tile([128, 512], F32, tag="ps2")
                                    pss.append(ps2)
                                for j in range(NJ):
                                    for tt in range(2):
                                        ti_loc = tg * 2 + tt
                                        nc.tensor.matmul(
                                            pss[tt][:],
                                            hT[:, j, ti_loc * 128:(ti_loc + 1) * 128],
                                            w2t[:, j, dh * 512:dh * 512 + 512],
                                            start=(j == 0), stop=False)
                                for tt in range(2):
                                    ti_loc = tg * 2 + tt
                                    ti = half * 4 + ti_loc
                                    nc.tensor.matmul(
                                        pss[tt][:], ones16[0:1, :],
                                        b2sb[0:1, e * HIDDEN + dh * 512:e * HIDDEN + dh * 512 + 512],
                                        start=False, stop=True)
                                    gsl = G[:, ti, e:e + 1]
                                    asl = acc[:, ti, dh * 512:dh * 512 + 512]
                                    if e == 0:
                                        nc.vector.tensor_scalar_mul(asl, pss[tt][:], gsl)
                                    else:
                                        nc.vector.scalar_tensor_tensor(
                                            asl, pss[tt][:], gsl, asl,
                                            op0=ALU.mult, op1=ALU.add)
        # ---- store ----
        for ti in range(NTI):
            nc.sync.dma_start(out[ti * 128:(ti + 1) * 128, :], acc[:, ti, :])

    nc.compile()
    return nc


def prep_weights(gate_w, gate_b, w1, b1, w2, b2):
    E, D, F = NUM_EXPERTS, HIDDEN, EXPERT_HIDDEN
    w1p = np.ascontiguousarray(
        w1.reshape(E, NKT, 128, NJ, 128).transpose(0, 3, 2, 1, 4).reshape(E, NJ, 128, NKT * 128)
    ).astype(np.float16)
    w2p = np.ascontiguousarray(w2.reshape(E, NJ, 128, D)).astype(np.float16)
    gwp = np.ascontiguousarray(gate_w.reshape(NKT, 128, E).transpose(1, 0, 2))
    gbp = np.ascontiguousarray(gate_b.reshape(1, E))
    b1p = np.ascontiguousarray(b1.reshape(E, NJ, 128).transpose(2, 0, 1).reshape(128, E * NJ))
    b2p = b2.astype(np.float16)
    return dict(w1p=w1p, w2p=w2p, gwp=gwp, gbp=gbp, b1p=b1p, b2p=b2p)


def make_in_maps(x, gate_w, gate_b, w1, b1, w2, b2):
    x2d = np.asarray(x, dtype=np.float32).reshape(-1, HIDDEN)
    shared = prep_weights(np.asarray(gate_w, np.float32), np.asarray(gate_b, np.float32),
                          np.asarray(w1, np.float32), np.asarray(b1, np.float32),
                          np.asarray(w2, np.float32), np.asarray(b2, np.float32))
    in_maps = []
    for c in range(N_CORES):
        xs = x2d[c * TC:(c + 1) * TC]
        in_maps.append({"xT": np.ascontiguousarray(xs.T), **shared})
    return in_maps


_CACHED_NC = None


def kernel(x, gate_w, gate_b, w1, b1, w2, b2):
    global _CACHED_NC
    from concourse.bass_utils import run_bass_kernel_spmd

    orig_shape = np.asarray(x).shape
    in_maps = make_in_maps(x, gate_w, gate_b, w1, b1, w2, b2)
    if _CACHED_NC is None:
        _CACHED_NC = build_nc()
    res = run_bass_kernel_spmd(_CACHED_NC, in_maps, core_ids=list(range(N_CORES)))
    out = np.concatenate([res.results[c]["out"] for c in range(N_CORES)], axis=0)
    out = out.reshape(orig_shape).astype(np.float32)
    aux_loss = np.asarray(0.0, dtype=np.float32)
    return (out, aux_loss)


# revision 11
# speedup vs baseline: 1.8216x; 1.7745x over previous
"""MoE sparse layer (dense all-expert compute, top-2 gating) on 8 TRN2 cores.

Sharding: token-parallel. x has 8192 tokens; each core processes 1024 tokens
against all 8 experts (weights replicated). No collectives.

Per-core device program (T=1024 tokens, D=1024, E=8, F=4096):
  gating:  logits[t,e] = x @ gate_w + gate_b   (fp32 matmuls, K-tiled)
           G[t,e] = softmax(logits)[t,e] * [e in top2(t)]   (DVE/ACT small ops)
  experts: for e in 0..7, token-halves of 512:
           hT[f,t] = gelu(w1[e].T-tiles @ xT + b1[e])       (fp16 MMs, ACT gelu)
           y[t,d]  = hT.T-tiles @ w2[e] + b2[e]             (fp16 MMs, K=1 bias MM)
           acc[t,d] (+)= G[t,e] * y[t,d]                    (DVE scalar_tensor_tensor)
Layouts are host-prepped so every DMA is a contiguous [128, X] panel.
"""
import numpy as np
from contextlib import ExitStack

import concourse.bacc as bacc_mod
import concourse.tile as tile
import concourse.mybir as mybir

F32 = mybir.dt.float32
F16 = mybir.dt.float16
AF = mybir.ActivationFunctionType
ALU = mybir.AluOpType

N_CORES = 8
HIDDEN = 1024          # D
NUM_EXPERTS = 8        # E
EXPERT_HIDDEN = 4096   # F
TOP_K = 2
TC = 1024              # tokens per core
NKT = HIDDEN // 128    # 8 K-tiles over D
NJ = EXPERT_HIDDEN // 128  # 32 f-tiles
NTI = TC // 128        # 8 token tiles


def build_nc(dbg=False, w1bufs=3, ps1bufs=2, ps2bufs=4, experts_on=True):
    nc = bacc_mod.Bacc("TRN2", target_bir_lowering=False, debug=False)

    xT = nc.dram_tensor("xT", [HIDDEN, TC], F32, kind="ExternalInput").ap()
    w1p = nc.dram_tensor("w1p", [NUM_EXPERTS, NJ, 128, NKT * 128], F16, kind="ExternalInput").ap()
    w2p = nc.dram_tensor("w2p", [NUM_EXPERTS, NJ, 128, HIDDEN], F16, kind="ExternalInput").ap()
    gwp = nc.dram_tensor("gwp", [128, NKT, NUM_EXPERTS], F32, kind="ExternalInput").ap()
    gbp = nc.dram_tensor("gbp", [1, NUM_EXPERTS], F32, kind="ExternalInput").ap()
    b1p = nc.dram_tensor("b1p", [128, NUM_EXPERTS * NJ], F32, kind="ExternalInput").ap()
    b2p = nc.dram_tensor("b2p", [NUM_EXPERTS, HIDDEN], F16, kind="ExternalInput").ap()
    out = nc.dram_tensor("out", [TC, HIDDEN], F32, kind="ExternalOutput").ap()
    if dbg:
        dbglg = nc.dram_tensor("dbglg", [128, NTI * NUM_EXPERTS], F32, kind="ExternalOutput").ap()
        dbgG = nc.dram_tensor("dbgG", [128, NTI * NUM_EXPERTS], F32, kind="ExternalOutput").ap()

    with tile.TileContext(nc) as tc, ExitStack() as ctx:
        # ---- persistent SBUF pools (bytes/partition) ----
        pers = ctx.enter_context(tc.tile_pool(name="pers", bufs=1))
        x16 = pers.tile([128, NKT, TC], F16, tag="x16")             # 16KB
        w2t = pers.tile([128, NJ, HIDDEN], F16, tag="w2t")          # 64KB
        acc = pers.tile([128, NTI, HIDDEN], F32, tag="acc")         # 32KB
        G = pers.tile([128, NTI, NUM_EXPERTS], F32, tag="G")        # 256B
        b1sb = pers.tile([128, NUM_EXPERTS * NJ], F32, tag="b1sb")  # 1KB
        gwsb = pers.tile([128, NKT, NUM_EXPERTS], F32, tag="gwsb")  # 256B
        gbsb = pers.tile([1, NUM_EXPERTS], F32, tag="gbsb")
        ones16 = pers.tile([1, 128], F16, tag="ones16")
        ones32 = pers.tile([1, 128], F32, tag="ones32")

        nc.sync.dma_start(b1sb[:], b1p)
        nc.sync.dma_start(gwsb[:], gwp)
        nc.sync.dma_start(gbsb[:], gbp)
        nc.gpsimd.memset(ones16[:], 1.0)
        nc.gpsimd.memset(ones32[:], 1.0)

        # ---- gating (transient pools; close before expert phase) ----
        with tc.tile_pool(name="gat", bufs=1) as gat, \
             tc.tile_pool(name="gtmp", bufs=10) as gtmp, \
             tc.tile_pool(name="gps", bufs=1, space="PSUM") as gpsp:
            gps = gpsp.tile([128, NTI, NUM_EXPERTS], F32, tag="gps")  # one bank
            xg = gat.tile([128, NKT, TC], F32, tag="xg")
            nc.sync.dma_start(xg[:], xT.rearrange("(kt p) t -> p kt t", p=128))
            nc.vector.tensor_copy(x16[:], xg[:])  # fp32 -> fp16 cast
            # NOTE: each token tile's accumulation group must be fully
            # sequential within the shared PSUM bank (start=True clears
            # has_written bank-wide).
            for ti in range(NTI):
                for kt in range(NKT):
                    nc.tensor.matmul(gps[:, ti, :], xg[:, kt, ti * 128:(ti + 1) * 128],
                                     gwsb[:, kt, :], start=(kt == 0), stop=False)
                nc.tensor.matmul(gps[:, ti, :], ones32[0:1, :], gbsb[0:1, :],
                                 start=False, stop=True)
            if dbg:
                lgcopy = gat.tile([128, NTI * NUM_EXPERTS], F32, tag="lgcopy")
                nc.vector.tensor_copy(lgcopy[:], gps[:, :, :])
                nc.sync.dma_start(dbglg, lgcopy[:])
            # softmax + top-2 mask per token tile
            for ti in range(NTI):
                lg = gps[:, ti, :]
                mx = gtmp.tile([128, 1], F32, tag="mx")
                nc.vector.reduce_max(mx[:], lg, axis=mybir.AxisListType.X)
                nmx = gtmp.tile([128, 1], F32, tag="nmx")
                nc.vector.tensor_scalar_mul(nmx[:], mx[:], -1.0)
                ex = gtmp.tile([128, NUM_EXPERTS], F32, tag="ex")
                s = gtmp.tile([128, 1], F32, tag="s")
                nc.scalar.activation(ex[:], lg, AF.Exp, bias=nmx[:], accum_out=s[:])
                r = gtmp.tile([128, 1], F32, tag="r")
                nc.vector.reciprocal(r[:], s[:])
                m1 = gtmp.tile([128, 1], F32, tag="m1")
                nc.vector.reduce_max(m1[:], ex[:], axis=mybir.AxisListType.X)
                eq = gtmp.tile([128, NUM_EXPERTS], F32, tag="eq")
                nc.vector.tensor_scalar(eq[:], ex[:], m1[:], None, op0=ALU.is_equal)
                ex2 = gtmp.tile([128, NUM_EXPERTS], F32, tag="ex2")
                nc.vector.scalar_tensor_tensor(ex2[:], eq[:], -1e9, ex[:],
                                               op0=ALU.mult, op1=ALU.add)
                m2 = gtmp.tile([128, 1], F32, tag="m2")
                nc.vector.reduce_max(m2[:], ex2[:], axis=mybir.AxisListType.X)
                gm = gtmp.tile([128, NUM_EXPERTS], F32, tag="gm")
                nc.vector.scalar_tensor_tensor(gm[:], ex[:], m2[:], ex[:],
                                               op0=ALU.is_ge, op1=ALU.mult)
                nc.vector.tensor_scalar_mul(G[:, ti, :], gm[:], r[:])

        if dbg:
            nc.sync.dma_start(dbgG, G[:, :, :])
        # ---- expert phase ----
        # hT pool opens after the gating pool closed (SBUF headroom); holds
        # BOTH token halves so w1 panels stream once per expert and mm1 runs
        # 16 consecutive MMs per 2-bank PSUM tile with a single big gelu.
        with tc.tile_pool(name="hTp", bufs=1) as hTp, \
             tc.tile_pool(name="w1pool", bufs=w1bufs) as w1pool, \
             tc.tile_pool(name="b2pool", bufs=2) as b2pool, \
             tc.tile_pool(name="ps1", bufs=ps1bufs, space="PSUM") as ps1p, \
             tc.tile_pool(name="ps2", bufs=ps2bufs, space="PSUM") as ps2p:
            hT = hTp.tile([128, NJ, TC], F16, tag="hT")  # 64KB/part
            if not experts_on:
                nc.vector.memset(acc[:], 0.0)
            for e in range(NUM_EXPERTS if experts_on else 0):
                nc.sync.dma_start(w2t[:], w2p[e].rearrange("j p d -> p j d"))
                b2sb = b2pool.tile([1, HIDDEN], F16, tag="b2sb")
                nc.sync.dma_start(b2sb[:], b2p[e].unsqueeze(0))
                # mm1: hT[f, :] = gelu(w1[e].T @ x + b1[e]) for all 1024 tokens
                for j in range(NJ):
                    w1pan = w1pool.tile([128, NKT * 128], F16, tag="w1pan")
                    nc.sync.dma_start(w1pan[:], w1p[e, j])
                    ps1 = ps1p.tile([128, TC], F32, tag="ps1")  # 2 banks
                    for half in range(2):
                        tb = half * 512
                        for kt in range(NKT):
                            nc.tensor.matmul(
                                ps1[:, tb:tb + 512], w1pan[:, kt * 128:(kt + 1) * 128],
                                x16[:, kt, tb:tb + 512],
                                start=(kt == 0), stop=(kt == NKT - 1))
                    nc.scalar.activation(hT[:, j, :], ps1[:], AF.Gelu,
                                         bias=b1sb[:, e * NJ + j:e * NJ + j + 1])
                # mm2: 33-MM same-bank accumulation runs, one (ti, dh) at a time
                for ti in range(NTI):
                    for dh in range(2):
                        ps2 = ps2p.tile([128, 512], F32, tag="ps2")
                        for j in range(NJ):
                            nc.tensor.matmul(
                                ps2[:], hT[:, j, ti * 128:(ti + 1) * 128],
                                w2t[:, j, dh * 512:dh * 512 + 512],
                                start=(j == 0), stop=False)
                        nc.tensor.matmul(
                            ps2[:], ones16[0:1, :], b2sb[0:1, dh * 512:dh * 512 + 512],
                            start=False, stop=True)
                        gsl = G[:, ti, e:e + 1]
                        asl = acc[:, ti, dh * 512:dh * 512 + 512]
                        if e == 0:
                            nc.vector.tensor_scalar_mul(asl, ps2[:], gsl)
                        else:
                            nc.vector.scalar_tensor_tensor(
                                asl, ps2[:], gsl, asl,
                                op0=ALU.mult, op1=ALU.add)
        # ---- store ----
        for ti in range(NTI):
            nc.sync.dma_start(out[ti * 128:(ti + 1) * 128, :], acc[:, ti, :])

    nc.compile()
    return nc


def prep_weights(gate_w, gate_b, w1, b1, w2, b2):
    E, D, F = NUM_EXPERTS, HIDDEN, EXPERT_HIDDEN
    w1p = np.ascontiguousarray(
        w1.reshape(E, NKT, 128, NJ, 128).transpose(0, 3, 2, 1, 4).reshape(E, NJ, 128, NKT * 128)
    ).astype(np.float16)
    w2p = np.ascontiguousarray(w2.reshape(E, NJ, 128, D)).astype(np.float16)
    gwp = np.ascontiguousarray(gate_w.reshape(NKT, 128, E).transpose(1, 0, 2))
    gbp = np.ascontiguousarray(gate_b.reshape(1, E))
    b1p = np.ascontiguousarray(b1.reshape(E, NJ, 128).transpose(2, 0, 1).reshape(128, E * NJ))
    b2p = b2.astype(np.float16)
    return dict(w1p=w1p, w2p=w2p, gwp=gwp, gbp=gbp, b1p=b1p, b2p=b2p)


def make_in_maps(x, gate_w, gate_b, w1, b1, w2, b2):
    x2d = np.asarray(x, dtype=np.float32).reshape(-1, HIDDEN)
    shared = prep_weights(np.asarray(gate_w, np.float32), np.asarray(gate_b, np.float32),
                          np.asarray(w1, np.float32), np.asarray(b1, np.float32),
                          np.asarray(w2, np.float32), np.asarray(b2, np.float32))
    in_maps = []
    for c in range(N_CORES):
        xs = x2d[c * TC:(c + 1) * TC]
        in_maps.append({"xT": np.ascontiguousarray(xs.T), **shared})
    return in_maps


_CACHED_NC = None


def kernel(x, gate_w, gate_b, w1, b1, w2, b2):
    global _CACHED_NC
    from concourse.bass_utils import run_bass_kernel_spmd

    orig_shape = np.asarray(x).shape
    in_maps = make_in_maps(x, gate_w, gate_b, w1, b1, w2, b2)
    if _CACHED_NC is None:
        _CACHED_NC = build_nc()
    res = run_bass_kernel_spmd(_CACHED_NC, in_maps, core_ids=list(range(N_CORES)))
    out = np.concatenate([res.results[c]["out"] for c in range(N_CORES)], axis=0)
    out = out.reshape(orig_shape).astype(np.float32)
    aux_loss = np.asarray(0.0, dtype=np.float32)
    return (out, aux_loss)
